# revision 13
# baseline (speedup 1.0000x reference)
"""VQ-VAE encoder kernel for Trainium2, data-parallel over 8 NeuronCores.

Per-core shard (4096 rows of x):
  phase 1: MLP 768->512->256->128 with transposed activations (features on
           partitions), all matmuls in float32r (full PE rate at N=512).
  phase 2: scores s_j = y.c_j + t_j where y = h2@W3 (b3 folded into
           t_j = b3.c_j - 0.5*||c_j||^2, precomputed on host).  argmax_j s_j
           == argmin_j ||z - c_j||^2, exact in fp32.
           Per 128-row chunk: 16 matmuls (z stationary, codebookT moving)
           -> PSUM; DVE tensor_tensor_reduce drains PSUM with the t-fold and
           per-512-chunk maxima; two half-row max_index ops recover the
           argmax position; tensor_mask_reduce selects the winning half/chunk
           per partition.  Codebook row gathered via indirect DMA.
"""

import numpy as np

import concourse.bacc as bacc
import concourse.bass as bass
import concourse.mybir as mybir
from concourse.bass_utils import run_bass_kernel_spmd
from concourse.tile import TileContext

N = 32768
SH = N // 8          # rows per core
D_IN = 768
H1 = 512
H2 = 256
D = 128
K = 8192
RB = 512             # MLP row-block
NRB = SH // RB       # 8
RC = 128             # distance row-chunk (partition dim)
NRC = SH // RC       # 32
CH = 512             # score chunk width (one PSUM bank)
NCH = K // CH        # 16

F32 = mybir.dt.float32
F32R = mybir.dt.float32r
U32 = mybir.dt.uint32
I32 = mybir.dt.int32
ALU = mybir.AluOpType
ACT_F = mybir.ActivationFunctionType
AXIS = mybir.AxisListType

NEG = -3.0e38


def build_nc(reps: int = 1) -> bass.Bass:
    nc = bacc.Bacc(None, target_bir_lowering=False, debug=False)

    xt = nc.declare_dram_parameter("xt", [D_IN, SH], F32, isOutput=False)
    w1 = nc.declare_dram_parameter("w1", [D_IN, H1], F32, isOutput=False)
    w2 = nc.declare_dram_parameter("w2", [H1, H2], F32, isOutput=False)
    w3 = nc.declare_dram_parameter("w3", [H2, D], F32, isOutput=False)
    b1 = nc.declare_dram_parameter("b1", [H1], F32, isOutput=False)
    b2 = nc.declare_dram_parameter("b2", [H2], F32, isOutput=False)
    cbt = nc.declare_dram_parameter("cbt", [D, K], F32, isOutput=False)
    trep = nc.declare_dram_parameter("trep", [128, K], F32, isOutput=False)
    cb = nc.declare_dram_parameter("cb", [K, D], F32, isOutput=False)
    quant = nc.declare_dram_parameter("quantized", [SH, D], F32, isOutput=True)
    codes_o = nc.declare_dram_parameter("codes", [128, NRC], I32, isOutput=True)

    with TileContext(nc) as tc:
        with tc.tile_pool(name="const", bufs=1) as cp:
            # ---- resident constants ----
            w1_t = []
            for k in range(6):
                t = cp.tile([128, H1], F32, tag=f"w1_{k}")
                nc.sync.dma_start(out=t[:], in_=w1[k * 128:(k + 1) * 128, :])
                w1_t.append(t)
            w2_t = []
            for k in range(4):
                t = cp.tile([128, H2], F32, tag=f"w2_{k}")
                nc.sync.dma_start(out=t[:], in_=w2[k * 128:(k + 1) * 128, :])
                w2_t.append(t)
            w3_t = []
            for k in range(2):
                t = cp.tile([128, D], F32, tag=f"w3_{k}")
                nc.sync.dma_start(out=t[:], in_=w3[k * 128:(k + 1) * 128, :])
                w3_t.append(t)
            b1_t = []
            b1_r = b1.rearrange("(c p one) -> c p one", p=128, one=1)
            for m in range(4):
                t = cp.tile([128, 1], F32, tag=f"b1_{m}")
                nc.sync.dma_start(out=t[:], in_=b1_r[m])
                b1_t.append(t)
            b2_t = []
            b2_r = b2.rearrange("(c p one) -> c p one", p=128, one=1)
            for m in range(2):
                t = cp.tile([128, 1], F32, tag=f"b2_{m}")
                nc.sync.dma_start(out=t[:], in_=b2_r[m])
                b2_t.append(t)
            cbt_t = cp.tile([128, K], F32, tag="cbt")
            nc.sync.dma_start(out=cbt_t[:], in_=cbt[:])
            trep_t = cp.tile([128, K], F32, tag="trep")
            nc.sync.dma_start(out=trep_t[:], in_=trep[:])

            yt = cp.tile([128, SH], F32, tag="yt")          # z minus b3, transposed
            s_sb = cp.tile([128, K], F32, tag="s")          # scores, one row-chunk
            codes_sb = cp.tile([128, NRC], I32, tag="codes")

            # ---- phase 1: MLP ----
            with (
                tc.tile_pool(name="mlp", bufs=2) as mp,
                tc.tile_pool(name="mps", bufs=4, space="PSUM") as mpp,
            ):
              for b in range(NRB):
                cols = bass.ts(b, RB)
                xk_t = []
                for k in range(6):
                    t = mp.tile([128, RB], F32, tag=f"x_{k}")
                    nc.sync.dma_start(out=t[:], in_=xt[k * 128:(k + 1) * 128, cols])
                    xk_t.append(t)
                h1_t = []
                for m in range(4):
                    ps = mpp.tile([128, RB], F32, tag="ps")
                    for k in range(6):
                        nc.tensor.matmul(
                            ps[:],
                            lhsT=w1_t[k][:, m * 128:(m + 1) * 128],
                            rhs=xk_t[k][:],
                            start=(k == 0), stop=(k == 5),
                        )
                    h = mp.tile([128, RB], F32, tag=f"h1_{m}")
                    nc.scalar.activation(h[:], ps[:], ACT_F.Relu, bias=b1_t[m][:], scale=1.0)
                    h1_t.append(h)
                h2_t = []
                for m in range(2):
                    ps = mpp.tile([128, RB], F32, tag="ps")
                    for k in range(4):
                        nc.tensor.matmul(
                            ps[:],
                            lhsT=w2_t[k][:, m * 128:(m + 1) * 128],
                            rhs=h1_t[k][:],
                            start=(k == 0), stop=(k == 3),
                        )
                    h = mp.tile([128, RB], F32, tag=f"h2_{m}")
                    nc.scalar.activation(h[:], ps[:], ACT_F.Relu, bias=b2_t[m][:], scale=1.0)
                    h2_t.append(h)
                ps = mpp.tile([128, RB], F32, tag="ps")
                for k in range(2):
                    nc.tensor.matmul(
                        ps[:],
                        lhsT=w3_t[k][:],
                        rhs=h2_t[k][:],
                        start=(k == 0), stop=(k == 1),
                    )
                nc.scalar.activation(yt[:, cols], ps[:], ACT_F.Copy, bias=0.0, scale=1.0)

            # ---- phase 2: scores + argmax + gather ----
            with (
                tc.tile_pool(name="dps", bufs=8, space="PSUM") as dpp,
                tc.tile_pool(name="sm", bufs=2) as sp,
                tc.tile_pool(name="qp", bufs=2) as qp,
            ):
             for _rep in range(reps):
              for rc in range(NRC):
                zc = yt[:, bass.ts(rc, RC)]
                for c in range(NCH):
                    ps = dpp.tile([128, CH], F32, tag="dps")
                    nc.tensor.matmul(
                        ps[:],
                        lhsT=zc,
                        rhs=cbt_t[:, bass.ts(c, CH)],
                        start=True, stop=True,
                    )
                    if c % 2 == 0:
                        # DVE: fused fold + PSUM->SBUF move
                        nc.vector.tensor_tensor(
                            out=s_sb[:, bass.ts(c, CH)], in0=ps[:],
                            in1=trep_t[:, bass.ts(c, CH)], op=ALU.add)
                    else:
                        # ACT copies raw; GPSIMD folds in SBUF
                        nc.scalar.activation(
                            s_sb[:, bass.ts(c, CH)], ps[:], ACT_F.Copy,
                            bias=0.0, scale=1.0)
                        nc.gpsimd.tensor_tensor(
                            out=s_sb[:, bass.ts(c, CH)],
                            in0=s_sb[:, bass.ts(c, CH)],
                            in1=trep_t[:, bass.ts(c, CH)], op=ALU.add)
                m8 = sp.tile([128, 8], F32, tag="m8")
                nc.vector.max(m8[:], s_sb[:])
                p8 = sp.tile([128, 8], U32, tag="p8")
                nc.vector.max_index(p8[:], m8[:], s_sb[:])
                code_u = sp.tile([128, 1], U32, tag="code_u")
                nc.vector.tensor_copy(out=code_u[:], in_=p8[:, 0:1])
                nc.vector.tensor_copy(out=codes_sb[:, rc:rc + 1], in_=p8[:, 0:1])
                # gather codebook rows for this chunk
                q = qp.tile([128, D], F32, tag="q")
                nc.gpsimd.indirect_dma_start(
                    out=q[:],
                    out_offset=None,
                    in_=cb[:],
                    in_offset=bass.IndirectOffsetOnAxis(ap=code_u[:], axis=0),
                )
                nc.sync.dma_start(out=quant[bass.ts(rc, RC), :], in_=q[:])

            nc.sync.dma_start(out=codes_o[:], in_=codes_sb[:])

    nc.compile()
    return nc


_NC_CACHE = None


def _get_nc():
    global _NC_CACHE
    if _NC_CACHE is None:
        _NC_CACHE = build_nc()
    return _NC_CACHE



def kernel(x, W1, b1, W2, b2, W3, b3, codebook):
    x = np.ascontiguousarray(np.asarray(x, np.float32))
    W1 = np.ascontiguousarray(np.asarray(W1, np.float32))
    W2 = np.ascontiguousarray(np.asarray(W2, np.float32))
    W3 = np.ascontiguousarray(np.asarray(W3, np.float32))
    b1 = np.ascontiguousarray(np.asarray(b1, np.float32))
    b2 = np.ascontiguousarray(np.asarray(b2, np.float32))
    b3 = np.asarray(b3, np.float64)
    cb = np.ascontiguousarray(np.asarray(codebook, np.float32))

    cbt = np.ascontiguousarray(cb.T)
    # t_j = b3.c_j - 0.5*||c_j||^2 (fp64 on host, cast to fp32)
    cb64 = cb.astype(np.float64)
    t = (cb64 @ b3 - 0.5 * (cb64 * cb64).sum(1)).astype(np.float32)
    trep = np.ascontiguousarray(np.broadcast_to(t[None, :], (128, K)))

    nc = _get_nc()
    in_maps = []
    for core in range(8):
        xs = x[core * SH:(core + 1) * SH]
        in_maps.append({
            "xt": np.ascontiguousarray(xs.T),
            "w1": W1, "w2": W2, "w3": W3,
            "b1": b1, "b2": b2,
            "cbt": cbt, "trep": trep, "cb": cb,
        })

    res = run_bass_kernel_spmd(nc, in_maps, list(range(8))).results

    quant = np.empty((N, D), np.float32)
    codes = np.empty((N,), np.int32)
    for core in range(8):
        quant[core * SH:(core + 1) * SH] = res[core]["quantized"]
        # codes dram is [128 partitions, 32 row-chunks]: row rc*128+p -> [p, rc]
        codes[core * SH:(core + 1) * SH] = res[core]["codes"].T.reshape(-1)
    return quant, codes


# revision 14
# speedup vs baseline: 164.3538x; 164.3538x over previous
"""VQ-VAE encoder kernel for Trainium2, data-parallel over 8 NeuronCores.

Per-core shard (4096 rows of x):
  phase 1: MLP 768->512->256->128 with transposed activations (features on
           partitions), all matmuls in float32r (full PE rate at N=512).
  phase 2: scores s_j = y.c_j + t_j where y = h2@W3 (b3 folded into
           t_j = b3.c_j - 0.5*||c_j||^2, precomputed on host).  argmax_j s_j
           == argmin_j ||z - c_j||^2, exact in fp32.
           Per 128-row chunk: 16 matmuls (z stationary, codebookT moving)
           -> PSUM; DVE tensor_tensor_reduce drains PSUM with the t-fold and
           per-512-chunk maxima; two half-row max_index ops recover the
           argmax position; tensor_mask_reduce selects the winning half/chunk
           per partition.  Codebook row gathered via indirect DMA.
"""

import numpy as np

import concourse.bacc as bacc
import concourse.bass as bass
import concourse.mybir as mybir
from concourse.bass_utils import run_bass_kernel_spmd
from concourse.tile import TileContext

N = 32768
SH = N // 8          # rows per core
D_IN = 768
H1 = 512
H2 = 256
D = 128
K = 8192
RB = 512             # MLP row-block
NRB = SH // RB       # 8
RC = 128             # distance row-chunk (partition dim)
NRC = SH // RC       # 32
CH = 512             # score chunk width (one PSUM bank)
NCH = K // CH        # 16

F32 = mybir.dt.float32
F32R = mybir.dt.float32r
U32 = mybir.dt.uint32
I32 = mybir.dt.int32
ALU = mybir.AluOpType
ACT_F = mybir.ActivationFunctionType
AXIS = mybir.AxisListType

NEG = -3.0e38


def build_nc(reps: int = 1) -> bass.Bass:
    nc = bacc.Bacc(None, target_bir_lowering=False, debug=False)

    xt = nc.declare_dram_parameter("xt", [D_IN, SH], F32, isOutput=False)
    w1 = nc.declare_dram_parameter("w1", [D_IN, H1], F32, isOutput=False)
    w2 = nc.declare_dram_parameter("w2", [H1, H2], F32, isOutput=False)
    w3 = nc.declare_dram_parameter("w3", [H2, D], F32, isOutput=False)
    b1 = nc.declare_dram_parameter("b1", [H1], F32, isOutput=False)
    b2 = nc.declare_dram_parameter("b2", [H2], F32, isOutput=False)
    cbt = nc.declare_dram_parameter("cbt", [D, K], F32, isOutput=False)
    trep = nc.declare_dram_parameter("trep", [128, K], F32, isOutput=False)
    cb = nc.declare_dram_parameter("cb", [K, D], F32, isOutput=False)
    quant = nc.declare_dram_parameter("quantized", [SH, D], F32, isOutput=True)
    codes_o = nc.declare_dram_parameter("codes", [128, NRC], I32, isOutput=True)

    with TileContext(nc) as tc:
        with tc.tile_pool(name="const", bufs=1) as cp:
            # ---- resident constants ----
            w1_t = []
            for k in range(6):
                t = cp.tile([128, H1], F32, tag=f"w1_{k}")
                nc.sync.dma_start(out=t[:], in_=w1[k * 128:(k + 1) * 128, :])
                w1_t.append(t)
            w2_t = []
            for k in range(4):
                t = cp.tile([128, H2], F32, tag=f"w2_{k}")
                nc.sync.dma_start(out=t[:], in_=w2[k * 128:(k + 1) * 128, :])
                w2_t.append(t)
            w3_t = []
            for k in range(2):
                t = cp.tile([128, D], F32, tag=f"w3_{k}")
                nc.sync.dma_start(out=t[:], in_=w3[k * 128:(k + 1) * 128, :])
                w3_t.append(t)
            b1_t = []
            b1_r = b1.rearrange("(c p one) -> c p one", p=128, one=1)
            for m in range(4):
                t = cp.tile([128, 1], F32, tag=f"b1_{m}")
                nc.sync.dma_start(out=t[:], in_=b1_r[m])
                b1_t.append(t)
            b2_t = []
            b2_r = b2.rearrange("(c p one) -> c p one", p=128, one=1)
            for m in range(2):
                t = cp.tile([128, 1], F32, tag=f"b2_{m}")
                nc.sync.dma_start(out=t[:], in_=b2_r[m])
                b2_t.append(t)
            cbt_t = cp.tile([128, K], F32, tag="cbt")
            nc.sync.dma_start(out=cbt_t[:], in_=cbt[:])
            trep_t = cp.tile([128, K], F32, tag="trep")
            nc.sync.dma_start(out=trep_t[:], in_=trep[:])

            yt = cp.tile([128, SH], F32, tag="yt")          # z minus b3, transposed
            s_sb = cp.tile([128, K], F32, tag="s")          # scores, one row-chunk
            codes_sb = cp.tile([128, NRC], I32, tag="codes")

            # ---- phase 1: MLP ----
            for _rep in range(reps):
             with (
                tc.tile_pool(name="mlp", bufs=2) as mp,
                tc.tile_pool(name="mps", bufs=4, space="PSUM") as mpp,
             ):
              for b in range(NRB):
                cols = bass.ts(b, RB)
                xk_t = []
                for k in range(6):
                    t = mp.tile([128, RB], F32, tag=f"x_{k}")
                    nc.sync.dma_start(out=t[:], in_=xt[k * 128:(k + 1) * 128, cols])
                    xk_t.append(t)
                h1_t = []
                for m in range(4):
                    ps = mpp.tile([128, RB], F32, tag="ps")
                    for k in range(6):
                        nc.tensor.matmul(
                            ps[:],
                            lhsT=w1_t[k][:, m * 128:(m + 1) * 128],
                            rhs=xk_t[k][:],
                            start=(k == 0), stop=(k == 5),
                        )
                    h = mp.tile([128, RB], F32, tag=f"h1_{m}")
                    nc.scalar.activation(h[:], ps[:], ACT_F.Relu, bias=b1_t[m][:], scale=1.0)
                    h1_t.append(h)
                h2_t = []
                for m in range(2):
                    ps = mpp.tile([128, RB], F32, tag="ps")
                    for k in range(4):
                        nc.tensor.matmul(
                            ps[:],
                            lhsT=w2_t[k][:, m * 128:(m + 1) * 128],
                            rhs=h1_t[k][:],
                            start=(k == 0), stop=(k == 3),
                        )
                    h = mp.tile([128, RB], F32, tag=f"h2_{m}")
                    nc.scalar.activation(h[:], ps[:], ACT_F.Relu, bias=b2_t[m][:], scale=1.0)
                    h2_t.append(h)
                ps = mpp.tile([128, RB], F32, tag="ps")
                for k in range(2):
                    nc.tensor.matmul(
                        ps[:],
                        lhsT=w3_t[k][:],
                        rhs=h2_t[k][:],
                        start=(k == 0), stop=(k == 1),
                    )
                nc.scalar.activation(yt[:, cols], ps[:], ACT_F.Copy, bias=0.0, scale=1.0)

             # ---- phase 2: scores + argmax + gather ----
             with (
                tc.tile_pool(name="dps", bufs=8, space="PSUM") as dpp,
                tc.tile_pool(name="sm", bufs=2) as sp,
                tc.tile_pool(name="qp", bufs=2) as qp,
             ):
              for rc in range(NRC):
                zc = yt[:, bass.ts(rc, RC)]
                for c in range(NCH):
                    ps = dpp.tile([128, CH], F32, tag="dps")
                    nc.tensor.matmul(
                        ps[:],
                        lhsT=zc,
                        rhs=cbt_t[:, bass.ts(c, CH)],
                        start=True, stop=True,
                    )
                    if c % 2 == 0:
                        # DVE: fused fold + PSUM->SBUF move
                        nc.vector.tensor_tensor(
                            out=s_sb[:, bass.ts(c, CH)], in0=ps[:],
                            in1=trep_t[:, bass.ts(c, CH)], op=ALU.add)
                    else:
                        # ACT copies raw; GPSIMD folds in SBUF
                        nc.scalar.activation(
                            s_sb[:, bass.ts(c, CH)], ps[:], ACT_F.Copy,
                            bias=0.0, scale=1.0)
                        nc.gpsimd.tensor_tensor(
                            out=s_sb[:, bass.ts(c, CH)],
                            in0=s_sb[:, bass.ts(c, CH)],
                            in1=trep_t[:, bass.ts(c, CH)], op=ALU.add)
                m8 = sp.tile([128, 8], F32, tag="m8")
                nc.vector.max(m8[:], s_sb[:])
                p8 = sp.tile([128, 8], U32, tag="p8")
                nc.vector.max_index(p8[:], m8[:], s_sb[:])
                code_u = sp.tile([128, 1], U32, tag="code_u")
                nc.vector.tensor_copy(out=code_u[:], in_=p8[:, 0:1])
                nc.vector.tensor_copy(out=codes_sb[:, rc:rc + 1], in_=p8[:, 0:1])
                # gather codebook rows for this chunk
                q = qp.tile([128, D], F32, tag="q")
                nc.gpsimd.indirect_dma_start(
                    out=q[:],
                    out_offset=None,
                    in_=cb[:],
                    in_offset=bass.IndirectOffsetOnAxis(ap=code_u[:], axis=0),
                )
                nc.sync.dma_start(out=quant[bass.ts(rc, RC), :], in_=q[:])

            nc.sync.dma_start(out=codes_o[:], in_=codes_sb[:])

    nc.compile()
    return nc


_NC_CACHE = None


def _get_nc():
    global _NC_CACHE
    if _NC_CACHE is None:
        _NC_CACHE = build_nc()
    return _NC_CACHE



def kernel(x, W1, b1, W2, b2, W3, b3, codebook):
    x = np.ascontiguousarray(np.asarray(x, np.float32))
    W1 = np.ascontiguousarray(np.asarray(W1, np.float32))
    W2 = np.ascontiguousarray(np.asarray(W2, np.float32))
    W3 = np.ascontiguousarray(np.asarray(W3, np.float32))
    b1 = np.ascontiguousarray(np.asarray(b1, np.float32))
    b2 = np.ascontiguousarray(np.asarray(b2, np.float32))
    b3 = np.asarray(b3, np.float64)
    cb = np.ascontiguousarray(np.asarray(codebook, np.float32))

    cbt = np.ascontiguousarray(cb.T)
    # t_j = b3.c_j - 0.5*||c_j||^2 (fp64 on host, cast to fp32)
    cb64 = cb.astype(np.float64)
    t = (cb64 @ b3 - 0.5 * (cb64 * cb64).sum(1)).astype(np.float32)
    trep = np.ascontiguousarray(np.broadcast_to(t[None, :], (128, K)))

    nc = _get_nc()
    in_maps = []
    for core in range(8):
        xs = x[core * SH:(core + 1) * SH]
        in_maps.append({
            "xt": np.ascontiguousarray(xs.T),
            "w1": W1, "w2": W2, "w3": W3,
            "b1": b1, "b2": b2,
            "cbt": cbt, "trep": trep, "cb": cb,
        })

    res = run_bass_kernel_spmd(nc, in_maps, list(range(8))).results

    quant = np.empty((N, D), np.float32)
    codes = np.empty((N,), np.int32)
    for core in range(8):
        quant[core * SH:(core + 1) * SH] = res[core]["quantized"]
        # codes dram is [128 partitions, 32 row-chunks]: row rc*128+p -> [p, rc]
        codes[core * SH:(core + 1) * SH] = res[core]["codes"].T.reshape(-1)
    return quant, codes
